# revision 17
# baseline (speedup 1.0000x reference)
"""Trainium2 Bass kernel for nn_CustomAttention (B=4, N=2048, DIM=1024, 16 heads x 64).

Sharding: 8 cores = 4 batches x 2 head-groups (8 heads each).
Per core: QKV projection for its 8 heads, attention, partial out-projection
(its 512 rows of w_out). Host sums the two partial outputs per batch + bias.

v2: all-bf16 single fused pipeline (fp32 PSUM accumulation throughout).
 - Host pre-converts inputs to bf16 (x transposed to [DIM, N]).
 - One long pipeline: V proj -> QK proj pair 0 -> per pair: attention with
   the NEXT pair's QK projection matmuls interleaved between attention
   chunks (fills PE while ACT paces exp); out-projection interleaved into
   pair 3's attention chunks.
 - exp on ACT: one [128,1024] instruction per key-tile covers both heads of
   a pair (their S tiles share one 2-bank PSUM allocation).
 - Softmax denominator via ones-column appended to V (row 64 of the O PSUM).
 - PSUM budget exactly 8 banks: S pair-tile 2x2, O 1+1, proj 2 (later y 2).
 - DMA queues: SP = weight/x input streams, Pool(gpsimd) = normalization
   row moves, DVE = y output stores.
"""

import contextlib
import sys

sys.path.insert(0, '/opt/trn_rl_repo')

import numpy as np
import ml_dtypes

import concourse.bass as bass
import concourse.tile as tile
from concourse import bacc, mybir
from concourse.bass_utils import run_bass_kernel_spmd

B, N_TOK, DIM = 4, 2048, 1024
HEADS_TOTAL, D_HEAD = 16, 64
G_HEADS = 8              # heads per core
PAIRS = G_HEADS // 2     # head pairs per core
INNER_G = G_HEADS * D_HEAD   # 512, inner slice per core
SCALE = D_HEAD ** -0.5
F32 = mybir.dt.float32
BF16 = mybir.dt.bfloat16

# gpsimd partition_broadcast reading directly from a partition-64 view
# (skips a DMA bounce through partition 0). CoreSim accepts it; HW does not
# (the ucode reads physical partition 0 regardless of the AP base), so this
# must stay False.
DIRECT_BCAST = False

_NC_CACHE = {}


def build_kernel(n_tok=N_TOK, repeat=1):
    nc = bacc.Bacc("TRN2")
    xt = nc.declare_dram_parameter("xt", [DIM, n_tok], BF16, isOutput=False)
    wq = nc.declare_dram_parameter("wq", [DIM, INNER_G], BF16, isOutput=False)
    wk = nc.declare_dram_parameter("wk", [DIM, INNER_G], BF16, isOutput=False)
    wv = nc.declare_dram_parameter("wv", [DIM, INNER_G], BF16, isOutput=False)
    wo = nc.declare_dram_parameter("wo", [INNER_G, DIM], BF16, isOutput=False)
    y = nc.declare_dram_parameter("y", [n_tok, DIM], F32, isOutput=True)

    KD = DIM // 128          # 8 contraction tiles for projections
    NTT = n_tok // 128       # 16 token tiles
    KT = n_tok // 128        # 16 key tiles
    NQC = n_tok // 512       # 4 q-chunks of 512
    QCW = 512

    with tile.TileContext(nc) as tc:
      with (tc.For_i(0, repeat, 1) if repeat > 1 else contextlib.nullcontext()):
        with tc.tile_pool(name="persist", bufs=1) as persist, \
             tc.tile_pool(name="att_s", bufs=2, space="PSUM") as att_s, \
             tc.tile_pool(name="att_o", bufs=1, space="PSUM") as att_o, \
             tc.tile_pool(name="etp", bufs=3) as etp, \
             tc.tile_pool(name="norm_sb", bufs=2) as norm_sb, \
             tc.tile_pool(name="wvp", bufs=1) as wvp, \
             tc.tile_pool(name="wqk", bufs=2) as wqk, \
             tc.tile_pool(name="ysb", bufs=3) as ysbp:

            # ---------- persistent tiles ----------
            xt_all = persist.tile([128, KD, n_tok], BF16, name="xt_all")
            xt_sb = [xt_all[:, i, :] for i in range(KD)]
            qT = [persist.tile([128, n_tok], BF16, name=f"qT{p}") for p in range(PAIRS)]
            kT = [persist.tile([128, n_tok], BF16, name=f"kT{p}") for p in range(PAIRS)]
            vb = [persist.tile([128, G_HEADS, D_HEAD + 1], BF16, name=f"vb{t}")
                  for t in range(NTT)]
            aT = [persist.tile([128, n_tok], BF16, name=f"aT{p}") for p in range(PAIRS)]
            wo_sb = [persist.tile([128, DIM], BF16, name=f"wo{j}") for j in range(PAIRS)]
            wv_all = wvp.tile([128, KD, INNER_G], BF16, name="wv_all")
            wv_sb = [wv_all[:, i, :] for i in range(KD)]

            # fused input loads: one DMA for all of wv, one per xt token-column
            # chunk (fewer HWDGE round-trips). xt chunk 0 rides the idle ACT
            # queue so it streams concurrently with wv.
            xt_r = xt.rearrange("(i p) n -> p i n", p=128)
            nc.scalar.dma_start(out=xt_all[:, :, 0:512], in_=xt_r[:, :, 0:512])
            nc.sync.dma_start(out=wv_all, in_=wv.rearrange("(i p) n -> p i n", p=128))
            for c in range(1, NQC):
                cs = slice(c * 512, (c + 1) * 512)
                nc.sync.dma_start(out=xt_all[:, :, cs], in_=xt_r[:, :, cs])

            ones8 = persist.tile([128, G_HEADS], BF16, name="ones8")
            nc.vector.memset(ones8, 1.0)
            # ones row on partition 64, for the tail's PE-side reciprocal
            # broadcast (f32r matmul: out[64, q] = ones[1,64].T @ recip[1, q])
            ones_row = persist.tile([D_HEAD + 1, D_HEAD], F32, name="ones_row")
            nc.vector.memset(ones_row[64:65, :], 1.0)

            # ---------- emission helpers ----------
            def emit_qk_dma(p, which):
                wt = wq if which == 0 else wk
                tiles = []
                for i in range(KD):
                    wti = wqk.tile([128, 128], BF16, tag=f"w{which}_{i}",
                                   name=f"w{p}_{which}_{i}")
                    nc.sync.dma_start(
                        out=wti,
                        in_=wt[i * 128:(i + 1) * 128, p * 128:(p + 1) * 128])
                    tiles.append(wti)
                return tiles

            def emit_qk_group(p, which, wtiles, qc, proj_ps):
                dst = qT[p] if which == 0 else kT[p]
                pqk = proj_ps.tile([128, QCW], F32, tag="pp", name=f"pqk{p}_{which}_{qc}")
                for i in range(KD):
                    nc.tensor.matmul(out=pqk, lhsT=wtiles[i],
                                     rhs=xt_sb[i][:, qc * QCW:(qc + 1) * QCW],
                                     start=(i == 0), stop=(i == KD - 1))
                nc.vector.tensor_copy(out=dst[:, qc * QCW:(qc + 1) * QCW], in_=pqk)

            def emit_v_group(t, proj_ps):
                vps = proj_ps.tile([128, INNER_G], F32, tag="pp", name=f"vps{t}")
                for i in range(KD):
                    nc.tensor.matmul(out=vps,
                                     lhsT=xt_sb[i][:, t * 128:(t + 1) * 128],
                                     rhs=wv_sb[i],
                                     start=(i == 0), stop=(i == KD - 1))
                nc.vector.tensor_copy(out=vb[t][:, :, D_HEAD], in_=ones8)
                nc.vector.tensor_copy(
                    out=vb[t][:, :, 0:D_HEAD],
                    in_=vps.rearrange("p (h d) -> p h d", h=G_HEADS))

            def emit_attention(p, qc, filler=()):
                """Attention for (pair, q-chunk). `filler` is a list of
                zero-arg closures (independent PE/DVE work) spread into the
                kt loop to fill PE latency stalls."""
                filler = list(filler)
                fpos = 0
                q0 = qc * QCW
                o_ps = [att_o.tile([D_HEAD + 1, QCW], F32, tag=f"o{hl}",
                                   name=f"o_{p}_{qc}_{hl}") for hl in range(2)]
                ets = {}
                OLAG = 2
                for kt_i in range(KT + OLAG):
                    if kt_i < KT:
                        sp = att_s.tile([128, 2 * QCW], F32, tag="sp",
                                        name=f"s_{p}_{qc}_{kt_i}")
                        for hl in range(2):
                            po = hl * 64
                            nc.tensor.matmul(
                                out=sp[:, hl * QCW:(hl + 1) * QCW],
                                lhsT=kT[p][po:po + 64, kt_i * 128:(kt_i + 1) * 128],
                                rhs=qT[p][po:po + 64, q0:q0 + QCW],
                                start=True, stop=True)
                        et = etp.tile([128, 2 * QCW], BF16, tag="et",
                                      name=f"e_{p}_{qc}_{kt_i}")
                        nc.scalar.activation(out=et, in_=sp,
                                             func=mybir.ActivationFunctionType.Exp,
                                             scale=SCALE)
                        ets[kt_i] = et
                    # interleave filler before the O matmuls: if O(ko) must
                    # still wait on its exp, the filler keeps PE fed
                    want = (len(filler) * (kt_i + 1)) // (KT + OLAG)
                    while fpos < want:
                        filler[fpos]()
                        fpos += 1
                    ko = kt_i - OLAG
                    if ko >= 0:
                        for hl in range(2):
                            nc.tensor.matmul(
                                out=o_ps[hl],
                                lhsT=vb[ko][:, 2 * p + hl, :],
                                rhs=ets[ko][:, hl * QCW:(hl + 1) * QCW],
                                start=(ko == 0), stop=(ko == KT - 1))
                        del ets[ko]
                while fpos < len(filler):
                    filler[fpos]()
                    fpos += 1
                # normalize by the ones-column row sums (row 64): reciprocal
                # straight from PSUM, evacuate O to SBUF (frees banks for the
                # next chunk), broadcast the reciprocal row, multiply.
                # The last chunk skips the evacuation copy and uses the idle
                # SP queue for its DMAs — shortest critical path into the
                # tail out-projection.
                last = (p == PAIRS - 1 and qc == NQC - 1)
                for hl in ((1, 0) if last else (0, 1)):
                    rt = norm_sb.tile([D_HEAD + 1, QCW], F32, tag=f"rt{hl}",
                                      name=f"rt_{p}_{qc}_{hl}")
                    nc.vector.reciprocal(out=rt[64:65, :], in_=o_ps[hl][64:65, :])
                    if last:
                        # PE is idle here: broadcast the reciprocal row with a
                        # f32r matmul against a ones column (lower latency than
                        # the DMA-bounce + gpsimd path); multiply straight from
                        # PSUM (no evacuation needed — program ends after this)
                        oc = o_ps[hl]
                        rb = att_s.tile([D_HEAD, QCW], F32, tag="sp",
                                        name=f"rbps_{qc}_{hl}")
                        nc.tensor.matmul(
                            out=rb,
                            lhsT=ones_row[64:65, :].bitcast(mybir.dt.float32r),
                            rhs=rt[64:65, :].bitcast(mybir.dt.float32r),
                            start=True, stop=True)
                    else:
                        oc = norm_sb.tile([D_HEAD + 1, QCW], F32, tag=f"oc{hl}",
                                          name=f"oc_{p}_{qc}_{hl}")
                        nc.vector.tensor_copy(out=oc, in_=o_ps[hl])
                        r0 = norm_sb.tile([1, QCW], F32, tag=f"r0{hl}",
                                          name=f"r0_{p}_{qc}_{hl}")
                        nc.gpsimd.dma_start(out=r0, in_=rt[64:65, :])
                        rb = norm_sb.tile([64, QCW], F32, tag=f"rb{hl}",
                                          name=f"rb_{p}_{qc}_{hl}")
                        nc.gpsimd.partition_broadcast(rb, r0)
                    if hl == 0:
                        nc.vector.tensor_mul(aT[p][0:64, q0:q0 + QCW],
                                             oc[0:64, :], rb)
                    else:
                        tmpb = norm_sb.tile([64, QCW], BF16, tag="tmpb",
                                            name=f"tmpb_{p}_{qc}")
                        nc.vector.tensor_mul(tmpb, oc[0:64, :], rb)
                        (nc.sync if last else nc.gpsimd).dma_start(
                            out=aT[p][64:128, q0:q0 + QCW], in_=tmpb)

            def qk_group_items(p, which, wtiles, qc2, proj_ps):
                """The 8 matmuls (+final copy) of one QK projection group,
                as individual filler closures."""
                dst = qT[p] if which == 0 else kT[p]
                state = {}

                def mk(i):
                    def f():
                        if i == 0:
                            state['pqk'] = proj_ps.tile(
                                [128, QCW], F32, tag="pp",
                                name=f"pqk{p}_{which}_{qc2}")
                        nc.tensor.matmul(
                            out=state['pqk'], lhsT=wtiles[i],
                            rhs=xt_sb[i][:, qc2 * QCW:(qc2 + 1) * QCW],
                            start=(i == 0), stop=(i == KD - 1))
                        if i == KD - 1:
                            nc.vector.tensor_copy(
                                out=dst[:, qc2 * QCW:(qc2 + 1) * QCW],
                                in_=state['pqk'])
                    return f
                return [mk(i) for i in range(KD)]

            def outproj_items(qc, y_ps):
                """Out-projection for token chunk qc as filler closures."""
                items = []
                for tt in range(4 * qc, 4 * qc + 4):
                    for dc in range(2):
                        state = {}

                        def mk(j, tt=tt, dc=dc, state=state):
                            def f():
                                if j == 0:
                                    state['yps'] = y_ps.tile(
                                        [128, 512], F32, tag="yp",
                                        name=f"y_{tt}_{dc}")
                                nc.tensor.matmul(
                                    out=state['yps'],
                                    lhsT=aT[j][:, tt * 128:(tt + 1) * 128],
                                    rhs=wo_sb[j][:, dc * 512:(dc + 1) * 512],
                                    start=(j == 0), stop=(j == PAIRS - 1))
                                if j == PAIRS - 1:
                                    ysb = ysbp.tile([128, 512], F32, tag="ysb",
                                                    name=f"ysb_{tt}_{dc}")
                                    nc.vector.tensor_copy(out=ysb,
                                                          in_=state['yps'])
                                    nc.sync.dma_start(
                                        out=y[tt * 128:(tt + 1) * 128,
                                              dc * 512:(dc + 1) * 512],
                                        in_=ysb)
                            return f
                        items.extend(mk(j) for j in range(PAIRS))
                return items

            # ---------- pipeline emission ----------
            with tc.tile_pool(name="proj_ps", bufs=2, space="PSUM") as proj_ps:
                for t in range(NTT):
                    emit_v_group(t, proj_ps)
                w0 = emit_qk_dma(0, 0)
                for qc in range(NQC):
                    emit_qk_group(0, 0, w0, qc, proj_ps)
                w1 = emit_qk_dma(0, 1)
                for qc in range(NQC):
                    emit_qk_group(0, 1, w1, qc, proj_ps)
                # wo is only needed by the out-projection — keep its DMAs
                # clear of the startup xt/wv burst
                for j in range(PAIRS):
                    nc.sync.dma_start(out=wo_sb[j], in_=wo[j * 128:(j + 1) * 128, :])

                for p in range(PAIRS - 1):
                    nxt_tiles = {}
                    for qc in range(NQC):
                        # next pair's projections ride inside this chunk
                        which = qc // 2
                        if qc % 2 == 0:
                            nxt_tiles[which] = emit_qk_dma(p + 1, which)
                        filler = (
                            qk_group_items(p + 1, which, nxt_tiles[which],
                                           2 * (qc % 2) + 0, proj_ps)
                            + qk_group_items(p + 1, which, nxt_tiles[which],
                                             2 * (qc % 2) + 1, proj_ps))
                        emit_attention(p, qc, filler)

            with tc.tile_pool(name="y_ps", bufs=2, space="PSUM") as y_ps:
                for qc in range(NQC):
                    # out-projection of the previous chunk rides inside this
                    # chunk (its aT needs the normalize chain to finish)
                    filler = outproj_items(qc - 1, y_ps) if qc > 0 else []
                    emit_attention(PAIRS - 1, qc, filler)
                for f in outproj_items(NQC - 1, y_ps):
                    f()

    nc.compile()
    return nc


def kernel(x, w_qkv, w_out, b_out):
    x = np.asarray(x, dtype=np.float32)
    w_qkv = np.asarray(w_qkv, dtype=np.float32)
    w_out = np.asarray(w_out, dtype=np.float32)
    b_out = np.asarray(b_out, dtype=np.float32)

    if N_TOK not in _NC_CACHE:
        _NC_CACHE[N_TOK] = build_kernel(N_TOK)
    nc = _NC_CACHE[N_TOK]

    core_ids = list(range(8))
    in_maps = _make_in_maps(x, w_qkv, w_out)
    res = run_bass_kernel_spmd(nc, in_maps, core_ids)
    out = np.empty((B, N_TOK, DIM), dtype=np.float32)
    for b in range(B):
        out[b] = res.results[2 * b]["y"] + res.results[2 * b + 1]["y"] + b_out
    return out


def _make_in_maps(x, w_qkv, w_out):
    bf = ml_dtypes.bfloat16
    in_maps = []
    for c in range(8):
        b, g = c // 2, c % 2
        sl = slice(g * INNER_G, (g + 1) * INNER_G)
        in_maps.append({
            "xt": np.ascontiguousarray(x[b].T).astype(bf),
            "wq": np.ascontiguousarray(w_qkv[:, 0 * DIM + sl.start:0 * DIM + sl.stop]).astype(bf),
            "wk": np.ascontiguousarray(w_qkv[:, 1 * DIM + sl.start:1 * DIM + sl.stop]).astype(bf),
            "wv": np.ascontiguousarray(w_qkv[:, 2 * DIM + sl.start:2 * DIM + sl.stop]).astype(bf),
            "wo": np.ascontiguousarray(w_out[sl]).astype(bf),
        })
    return in_maps
